# revision 1
# baseline (speedup 1.0000x reference)
"""Deformable bilinear sampling TRN2 kernel, hybrid gather version.

w in [0,96): DMA-gather pipeline (as kernel.py). w in [96,128): Pool
ap_gather from an SBUF-resident f32 column-slab of the padded image,
weights broadcast across channel partitions via PE (transpose of the
natural weight tile + 16-contraction selector matmuls into PSUM), then
DVE mult+adds. Moves ~23% of gather traffic off the saturated DMA device.
"""

import numpy as np

import concourse.bacc as bacc
import concourse.bass as bass
import concourse.mybir as mybir
from concourse.library_config import mlp

PAIRS = 4
H = W = 128
C = 32
PAD = 8
HP = 144
NROWS = HP * HP
W_D = 96                 # dma-half columns
CH = 4
WCH = W_D // CH          # 24
NIDX_CH = H * WCH        # 3072
NCHUNK = PAIRS * CH      # 16
W_P = W - W_D            # 32 pool columns
PP = H * W_P             # 4096 pool pixels per pair
SLABC = 48               # slab cols (anchor cols 96..143)
NE2 = HP * SLABC         # 6912
PRND = 8                 # pool combine rounds (512 px/pair each)
PCH = PP // PRND         # 512

F32 = mybir.dt.float32
BF16 = mybir.dt.bfloat16
I16 = mybir.dt.int16
OP = mybir.AluOpType
TWO23 = 12582912.0


def build_nc(combine_split=None):
    if combine_split is None:
        combine_split = ["v"] * NCHUNK
        for s in (10, 11, 12, 13, 14):
            combine_split[s] = "g"
    nc = bacc.Bacc("TRN2")
    patches = nc.declare_dram_parameter("patches", [PAIRS, NROWS, 128], BF16, isOutput=False)
    offn = nc.declare_dram_parameter("offn", [PAIRS, 2, H, W], F32, isOutput=False)
    basen = nc.declare_dram_parameter("basen", [H, W_D], F32, isOutput=False)
    basen2 = nc.declare_dram_parameter("basen2", [H, W_P], F32, isOutput=False)
    img2 = nc.declare_dram_parameter("img2", [128, NE2], F32, isOutput=False)
    idn_d = nc.declare_dram_parameter("idn_d", [128, 128], BF16, isOutput=False)
    A_d = nc.declare_dram_parameter("A_d", [16, 4, 128], BF16, isOutput=False)
    out = nc.declare_dram_parameter("out", [PAIRS, H, W_D, C], BF16, isOutput=True)
    out_p = nc.declare_dram_parameter("out_p", [128, PP], BF16, isOutput=True)
    wtT_dram = nc.declare_dram_parameter("wtT_dram", [PAIRS, 128, 128], BF16, isOutput=True)

    from contextlib import ExitStack

    with ExitStack() as stack:
        ec = stack.enter_context
        block = ec(nc.Block())
        NG = 8
        NA = 6
        Gb = [ec(nc.sbuf_tensor(f"G{i}", [128, WCH, 128], BF16)) for i in range(NG)]
        accb = [ec(nc.sbuf_tensor(f"acc{i}", [128, WCH, C], BF16)) for i in range(NA)]
        tmpv = ec(nc.sbuf_tensor("tmpv", [128, WCH, C], BF16))
        tmpg = ec(nc.sbuf_tensor("tmpg", [128, WCH, C], BF16))
        onb = [ec(nc.sbuf_tensor(f"on{p}", [128, 2, W], F32)) for p in range(PAIRS)]
        bnat = ec(nc.sbuf_tensor("bnat", [128, W_D], F32))
        bnat2 = ec(nc.sbuf_tensor("bnat2", [128, W_P], F32))
        db = [ec(nc.sbuf_tensor(f"d{p}", [128, NIDX_CH * CH // 16], I16)) for p in range(PAIRS)]
        wtb = [ec(nc.sbuf_tensor(f"wt{p}", [128, 4, W], BF16)) for p in range(PAIRS)]
        wt2b = [ec(nc.sbuf_tensor(f"wt2{p}", [128, 4, W_P], BF16)) for p in range(PAIRS)]
        dnatb = [ec(nc.sbuf_tensor(f"dnat{p}", [128, W_D], I16)) for p in range(PAIRS)]
        dnat2b = [ec(nc.sbuf_tensor(f"dnat2{p}", [128, 4, W_P], I16)) for p in range(PAIRS)]
        sf = ec(nc.sbuf_tensor("sf", [128, 2, W], F32))
        sg = ec(nc.sbuf_tensor("sg", [128, 2, W], F32))
        sy2 = ec(nc.sbuf_tensor("sy2", [128, 2, W], F32))
        tD = ec(nc.sbuf_tensor("tD", [128, W_D], F32))
        tD2 = ec(nc.sbuf_tensor("tD2", [128, W_P], F32))
        imgs = ec(nc.sbuf_tensor("imgs", [128, NE2], F32))
        Gp = ec(nc.sbuf_tensor("Gp", [128, 4, PP], F32))
        dpool = ec(nc.sbuf_tensor("dpool", [128, 4 * PP // 16], I16))
        idn = ec(nc.sbuf_tensor("idn", [128, 128], BF16))
        A_sb = ec(nc.sbuf_tensor("A_sb", [16, 4, 128], BF16))
        Bsb = ec(nc.sbuf_tensor("Bsb", [16, PP], BF16))
        wtT = [ec(nc.sbuf_tensor(f"wtT{i}", [128, 128], BF16)) for i in range(2)]
        Gpb = [ec(nc.sbuf_tensor(f"Gpb{i}", [128, 4, PCH], BF16)) for i in range(2)]
        wxb = [ec(nc.sbuf_tensor(f"wxb{i}", [128, 4, PCH], BF16)) for i in range(2)]
        tmp4 = ec(nc.sbuf_tensor("tmp4", [128, 4, PCH], BF16))
        Ob = [ec(nc.sbuf_tensor(f"O{i}", [128, PCH], BF16)) for i in range(3)]
        psum_t = [ec(nc.psum_tensor(f"pt{i}", [128, 128], BF16)) for i in range(2)]
        wx = ec(nc.psum_tensor("wx", [128, 4, PCH], F32))

        s_inb = ec(nc.semaphore("s_inb"))
        s_inx = [ec(nc.semaphore(f"s_inx{p}")) for p in range(PAIRS)]
        s_inb2 = ec(nc.semaphore("s_inb2"))
        s_cA2 = ec(nc.semaphore("s_cA2"))
        s_img = ec(nc.semaphore("s_img"))
        s_cA = ec(nc.semaphore("s_cA"))
        s_g = [ec(nc.semaphore(f"s_g{i}")) for i in range(NCHUNK)]
        s_cmb = [ec(nc.semaphore(f"s_cmb{i}")) for i in range(NCHUNK)]
        s_out = [ec(nc.semaphore(f"s_out{i}")) for i in range(NCHUNK)]
        s_dn = ec(nc.semaphore("s_dn"))
        s_dn2 = ec(nc.semaphore("s_dn2"))
        s_wt = ec(nc.semaphore("s_wt"))
        s_dw = ec(nc.semaphore("s_dw"))
        s_dpw = ec(nc.semaphore("s_dpw"))
        s_pt = ec(nc.semaphore("s_pt"))
        s_ptc = ec(nc.semaphore("s_ptc"))
        s_bf = [ec(nc.semaphore(f"s_bf{i}")) for i in range(PAIRS)]
        s_b1 = [ec(nc.semaphore(f"s_b1{i}")) for i in range(PAIRS)]
        s_apg = ec(nc.semaphore("s_apg"))
        s_wx = ec(nc.semaphore("s_wx"))
        s_prm = ec(nc.semaphore("s_prm"))
        s_wxc = ec(nc.semaphore("s_wxc"))
        s_gpb = ec(nc.semaphore("s_gpb"))
        s_pr = ec(nc.semaphore("s_pr"))
        s_pout = [ec(nc.semaphore(f"s_pout{i}")) for i in range(PRND)]
        s_cv = ec(nc.semaphore("s_cv"))
        s_cg = ec(nc.semaphore("s_cg"))

        @block.sync
        def _(sync: bass.BassEngine):
            sync.dma_start(bnat[:, :], basen[:, :]).then_inc(s_inb, 16)
            sync.dma_start(bnat2[:, :], basen2[:, :]).then_inc(s_inb2, 16)
            for p in range(PAIRS):
                sync.dma_start(onb[p][:, :, :], offn[p, :, :, :].transpose([1, 0, 2])).then_inc(s_inx[p], 16)
            sync.dma_start(idn[:, :], idn_d[:, :]).then_inc(s_cA, 16)
            sync.dma_start(A_sb[:, :, :], A_d[:, :, :]).then_inc(s_cA2, 16)
            sync.dma_start(imgs[:, :], img2[:, :]).then_inc(s_img, 16)
            # output DMAs; B-flats and pool outs woven in
            def emit_flats(p):
                # bounce through DRAM: partition-parallel out, contiguous back
                sync.wait_ge(s_ptc, p + 1)
                sync.dma_start(wtT_dram[p, :, :], wtT[p % 2][:, :]).then_inc(s_b1[p], 16)
                sync.wait_ge(s_b1[p], 16)
                sync.dma_start(
                    Bsb[4 * p:4 * p + 4, :],
                    wtT_dram[p, :, :].rearrange("(k w) h -> k (w h)", k=4),
                ).then_inc(s_bf[p], 16)

            emit_flats(0)
            pout_next = 0
            for s in range(NCHUNK):
                if s in (2, 4, 6):
                    emit_flats(s // 2)
                sync.wait_ge(s_cmb[s], 1)
                p, c = divmod(s, CH)
                dst = out[p, :, c * WCH:(c + 1) * WCH, :]
                sync.dma_start(dst, accb[s % NA][:, :, :]).then_inc(s_out[s], 16)
                if s >= 9 and pout_next < 2 * (s - 8):
                    r = pout_next
                    sync.wait_ge(s_pr, r + 1)
                    sync.dma_start(out_p[:, r * PCH:(r + 1) * PCH], Ob[r % 3][:, :]).then_inc(s_pout[r], 16)
                    pout_next += 1
            for r in range(pout_next, PRND):
                sync.wait_ge(s_pr, r + 1)
                sync.dma_start(out_p[:, r * PCH:(r + 1) * PCH], Ob[r % 3][:, :]).then_inc(s_pout[r], 16)

        class Chain:
            def __init__(self, eng, sem):
                self.eng, self.sem, self.n = eng, sem, 0
                self.extra = []

            def run(self, thunk, final=None):
                if self.n:
                    self.eng.wait_ge(self.sem, self.n)
                for sem, val in self.extra:
                    self.eng.wait_ge(sem, val)
                self.extra = []
                inst = thunk()
                if final is None:
                    inst.then_inc(self.sem, 1)
                    self.n += 1
                else:
                    sem, val = final
                    inst.then_inc(sem, 1)
                    self.extra.append((sem, val))
                return inst

        def idx_weights(eng, ch, p):
            onf = onb[p][:, :, :]
            r = ch.run
            wt = wtb[p]
            r(lambda: eng.tensor_scalar(sy2[:, :, :], onf, TWO23, -TWO23, OP.add, OP.add))
            r(lambda: eng.tensor_tensor(sf[:, :, :], sy2[:, :, :], onf, OP.is_gt))
            r(lambda: eng.tensor_sub(sy2[:, :, :], sy2[:, :, :], sf[:, :, :]))
            r(lambda: eng.tensor_sub(sf[:, :, :], onf, sy2[:, :, :]))
            # pool anchors first (they gate ap_gather via ACT wrap)
            r(lambda: eng.scalar_tensor_tensor(tD2[:, :], sy2[:, 0, W_D:], float(SLABC), sy2[:, 1, W_D:], OP.mult, OP.add))
            r(lambda: eng.tensor_add(tD2[:, :], tD2[:, :], bnat2[:, :]))
            for k, off in enumerate((0.0, 1.0, float(SLABC), float(SLABC + 1))):
                kk, oo = k, off
                fin = (s_dn2, p + 1) if k == 3 else None
                r(lambda: eng.tensor_scalar(dnat2b[p][:, kk, :], tD2[:, :], 1.0, oo, OP.mult, OP.add), final=fin)
            # dma-half anchors (cols 0..W_D)
            r(lambda: eng.scalar_tensor_tensor(tD[:, :], sy2[:, 0, :W_D], float(HP), sy2[:, 1, :W_D], OP.mult, OP.add))
            r(lambda: eng.tensor_add(tD[:, :], tD[:, :], bnat[:, :]))
            r(lambda: eng.tensor_copy(dnatb[p][:, :], tD[:, :]), final=(s_dn, p + 1))
            # weights
            r(lambda: eng.tensor_scalar(sg[:, :, :], sf[:, :, :], -1.0, 1.0, OP.mult, OP.add))
            wt2 = wt2b[p]
            r(lambda: eng.tensor_mul(wt2[:, 0, :], sg[:, 0, W_D:], sg[:, 1, W_D:]))
            r(lambda: eng.tensor_mul(wt2[:, 1, :], sg[:, 0, W_D:], sf[:, 1, W_D:]))
            r(lambda: eng.tensor_mul(wt2[:, 2, :], sf[:, 0, W_D:], sg[:, 1, W_D:]))
            r(lambda: eng.tensor_mul(wt2[:, 3, :], sf[:, 0, W_D:], sf[:, 1, W_D:]), final=(s_wt, 2 * p + 1))
            r(lambda: eng.tensor_mul(wt[:, 0, :W_D], sg[:, 0, :W_D], sg[:, 1, :W_D]))
            r(lambda: eng.tensor_mul(wt[:, 1, :W_D], sg[:, 0, :W_D], sf[:, 1, :W_D]))
            r(lambda: eng.tensor_mul(wt[:, 2, :W_D], sf[:, 0, :W_D], sg[:, 1, :W_D]))
            r(lambda: eng.tensor_mul(wt[:, 3, :W_D], sf[:, 0, :W_D], sf[:, 1, :W_D]), final=(s_wt, 2 * p + 2))

        def emit_combine(eng, ch, s, tmp):
            p, c = divmod(s, CH)
            G = Gb[s % NG]
            acc = accb[s % NA]
            gflat = G[:, :, :]
            wt = wtb[p]
            ws = c * WCH
            r = ch.run

            def gk(k):
                return gflat[:, :, k * C:(k + 1) * C]

            def wk(k):
                a = wt[:, k, ws:ws + WCH]
                return a.unsqueeze(2).broadcast_to([128, WCH, C])

            r(lambda: eng.tensor_mul(acc[:, :, :], gk(0), wk(0)))
            r(lambda: eng.tensor_mul(tmp[:, :, :], gk(1), wk(1)))
            r(lambda: eng.tensor_add(acc[:, :, :], acc[:, :, :], tmp[:, :, :]))
            r(lambda: eng.tensor_mul(tmp[:, :, :], gk(2), wk(2)))
            r(lambda: eng.tensor_add(acc[:, :, :], acc[:, :, :], tmp[:, :, :]))
            r(lambda: eng.tensor_mul(tmp[:, :, :], gk(3), wk(3)))
            return lambda final: r(
                lambda: eng.tensor_add(acc[:, :, :], acc[:, :, :], tmp[:, :, :]),
                final=final,
            )

        @block.vector
        def _(vector: bass.BassEngine):
            ch = Chain(vector, s_cv)
            for p in range(PAIRS):
                ch.run(lambda p=p: vector.memset(db[p][:, :].bitcast(mybir.dt.uint32), 0))
            vector.wait_ge(s_inb, 16)
            vector.wait_ge(s_inb2, 16)
            for p in range(PAIRS):
                vector.wait_ge(s_inx[p], 16)
                idx_weights(vector, ch, p)

            def pool_round(r):
                vector.wait_ge(s_wx, r + 1)
                vector.wait_ge(s_apg, 1)
                if r >= 3:
                    vector.wait_ge(s_pout[r - 3], 16)
                vector.wait_ge(s_wxc, r + 1)
                vector.wait_ge(s_gpb, r + 1)
                ch.run(lambda: vector.tensor_mul(tmp4[:, :, :], Gpb[r % 2][:, :, :], wxb[r % 2][:, :, :]),
                       final=(s_prm, r + 1))
                ch.run(lambda: vector.tensor_add(tmp4[:, 0:2, :], tmp4[:, 0:2, :], tmp4[:, 2:4, :]))
                ch.run(lambda: vector.tensor_add(Ob[r % 3][:, :], tmp4[:, 0, :], tmp4[:, 1, :]),
                       final=(s_pr, r + 1))

            dve_chunks = [s for s in range(NCHUNK) if combine_split[s] == "v"]
            # weave: first 10 dma chunks, then alternate pool rounds
            plan = []
            pr = 0
            for i, s in enumerate(dve_chunks):
                plan.append(("c", s))
                if i >= 9 and pr < PRND:
                    plan.append(("r", pr))
                    pr += 1
            while pr < PRND:
                plan.append(("r", pr))
                pr += 1
            for kind, v in plan:
                if kind == "c":
                    vector.wait_ge(s_g[v], 16)
                    if v >= NA:
                        vector.wait_ge(s_out[v - NA], 16)
                    emit_combine(vector, ch, v, tmpv)((s_cmb[v], 1))
                else:
                    pool_round(v)

        @block.scalar
        def _(act: bass.BassEngine):
            for p in range(PAIRS):
                if p >= 1:
                    act.wait_ge(s_dw, 84 * p)
                    act.wait_ge(s_dpw, 84 * p)
                # pool wrap first: it gates ap_gather, the longest pole
                act.wait_ge(s_dn2, p + 1)
                dpv = dpool[32 * p:32 * p + 32, :].rearrange("p (c w k) -> p c w k", k=8, w=W_P)
                for k in range(0, 8, 2):
                    act.copy(dpv[0:16, :, :, k],
                             dnat2b[p][16 * k:16 * (k + 1), :, :]).then_inc(s_dpw, 1)
                with nc.allow_non_contiguous_dma(reason="pool idx-wrap strided dst"):
                    for k in range(1, 8, 2):
                        act.dma_start(dpv[0:16, :, :, k],
                                      dnat2b[p][16 * k:16 * (k + 1), :, :]).then_inc(s_dpw, 16)
                act.wait_ge(s_dpw, 84 * p + 68)
                act.dma_start(dpool[32 * p + 16:32 * p + 32, :], dpool[32 * p:32 * p + 16, :]).then_inc(s_dpw, 16)
                act.wait_ge(s_dn, p + 1)
                dwrap = db[p][:, :].rearrange("p (w k) -> p w k", k=8)
                for k in range(0, 8, 2):
                    act.copy(dwrap[0:16, :, k],
                             dnatb[p][16 * k:16 * (k + 1), :]).then_inc(s_dw, 1)
                with nc.allow_non_contiguous_dma(reason="idx-wrap strided dst"):
                    for k in range(1, 8, 2):
                        act.dma_start(dwrap[0:16, :, k],
                                      dnatb[p][16 * k:16 * (k + 1), :]).then_inc(s_dw, 16)
                act.wait_ge(s_dw, 84 * p + 68)
                act.dma_start(db[p][16:32, :], db[p][0:16, :]).then_inc(s_dw, 16)
                # wtT copy for PE-produced transpose
                act.wait_ge(s_pt, p + 1)
                if p >= 2:
                    act.wait_ge(s_b1[p - 2], 16)
                act.copy(wtT[p % 2][:, :], psum_t[p % 2][:, :]).then_inc(s_ptc, 1)
            for r in range(PRND):
                act.wait_ge(s_wx, r + 1)
                if r >= 2:
                    act.wait_ge(s_prm, r - 1)
                act.copy(wxb[r % 2][:, :, :], wx[:, :, :]).then_inc(s_wxc, 1)
                if r == 0:
                    act.wait_ge(s_apg, 1)
                act.copy(Gpb[r % 2][:, :, :], Gp[:, :, r * PCH:(r + 1) * PCH]).then_inc(s_gpb, 1)

        @block.tensor
        def _(pe: bass.BassEngine):
            pe.wait_ge(s_cA, 16)
            pe.wait_ge(s_cA2, 16)
            for p in range(PAIRS):
                pe.wait_ge(s_wt, 2 * p + 1)
                if p >= 2:
                    pe.wait_ge(s_ptc, p - 1)
                pe.transpose(psum_t[p % 2][:, :],
                             wt2b[p][:, :, :],
                             idn[:, :]).then_inc(s_pt, 1)
            for i in range(PAIRS):
                pe.wait_ge(s_bf[i], 16)
            for r in range(PRND):
                if r >= 1:
                    pe.wait_ge(s_wxc, r)
                for k in range(4):
                    inst = pe.matmul(wx[:, k, :], A_sb[:, k, :],
                                     Bsb[:, r * PCH:(r + 1) * PCH],
                                     start=True, stop=True)
                    if k == 3:
                        inst.then_inc(s_wx, 1)

        @block.gpsimd
        def _(gpsimd: bass.BassGpSimd):
            chg = Chain(gpsimd, s_cg)
            gpsimd.load_library(mlp)
            for s in range(NCHUNK):
                p, c = divmod(s, CH)
                if s == 10:
                    gpsimd.wait_ge(s_dpw, 84 * PAIRS)
                    gpsimd.wait_ge(s_img, 16)
                    gpsimd.ap_gather(Gp[:, :, :], imgs[:, :],
                                     dpool[:, :], 128, NE2, 1, 4 * PP).then_inc(s_apg, 1)
                gpsimd.wait_ge(s_dw, 84 * (p + 1))
                if s >= NG:
                    gpsimd.wait_ge(s_cmb[s - NG], 1)
                gpsimd.dma_gather(
                    Gb[s % NG][:, :, :],
                    patches[p, :, :],
                    db[p][:, c * (NIDX_CH // 16):(c + 1) * (NIDX_CH // 16)],
                    NIDX_CH,
                    NIDX_CH,
                    128,
                    single_packet=False,
                ).then_inc(s_g[s], 16)
            for s in range(NCHUNK):
                if combine_split[s] == "g":
                    gpsimd.wait_ge(s_wt, 2 * (s // CH) + 2)
                    gpsimd.wait_ge(s_g[s], 16)
                    if s >= NA:
                        gpsimd.wait_ge(s_out[s - NA], 16)
                    emit_combine(gpsimd, chg, s, tmpg)((s_cmb[s], 1))

    nc.compile()
    return nc


# ---------------- host-side helpers ----------------

def build_patches_all(imgs_pairs):
    import ml_dtypes

    npair = imgs_pairs.shape[0]
    hw_c = np.ascontiguousarray(np.transpose(imgs_pairs, (0, 2, 3, 1)))
    padded = np.zeros((npair, HP + 1, HP + 1, C), np.float32)
    padded[:, PAD:PAD + H, PAD:PAD + W] = hw_c
    P = np.empty((npair, HP, HP, 4, C), np.float32)
    P[:, :, :, 0] = padded[:, 0:HP, 0:HP]
    P[:, :, :, 1] = padded[:, 0:HP, 1:HP + 1]
    P[:, :, :, 2] = padded[:, 1:HP + 1, 0:HP]
    P[:, :, :, 3] = padded[:, 1:HP + 1, 1:HP + 1]
    return P.reshape(npair, NROWS, 128).astype(ml_dtypes.bfloat16)


def base_natural():
    h = np.arange(H).reshape(H, 1)
    w = np.arange(W_D).reshape(1, W_D)
    return ((h + PAD) * HP + (w + PAD)).astype(np.float32)


def base_natural2():
    h = np.arange(H).reshape(H, 1)
    w = np.arange(W_P).reshape(1, W_P)
    return ((h + PAD) * SLABC + (w + PAD)).astype(np.float32)


def build_img2(imgs_pairs):
    npair = imgs_pairs.shape[0]
    pad = np.zeros((npair, C, HP, HP), np.float32)
    pad[:, :, PAD:PAD + H, PAD:PAD + W] = imgs_pairs
    slab = pad[:, :, :, W_D:W_D + SLABC]          # (4, 32, 144, 48)
    return np.ascontiguousarray(slab.reshape(128, NE2))


def build_A():
    import ml_dtypes

    A = np.zeros((16, 4, 128), np.float32)
    for k in range(4):
        for m in range(128):
            A[(m // 32) * 4 + k, k, m] = 1.0
    return A.astype(ml_dtypes.bfloat16)


def make_in_map(imgs_pairs, offp):
    import ml_dtypes

    return {
        "patches": build_patches_all(imgs_pairs),
        "offn": np.ascontiguousarray(offp),
        "basen": base_natural(),
        "basen2": base_natural2(),
        "img2": build_img2(imgs_pairs),
        "idn_d": np.eye(128, dtype=np.float32).astype(ml_dtypes.bfloat16),
        "A_d": build_A(),
    }


# ---------------- public entry point ----------------

N_CORES = 8
PAIRS_TOTAL = 32

LAST_EXEC_TIME_NS = None


def kernel(images, offsets):
    import os
    global LAST_EXEC_TIME_NS
    from concourse.bass_utils import run_bass_kernel_spmd

    images = np.ascontiguousarray(np.asarray(images, dtype=np.float32))
    offsets = np.ascontiguousarray(np.asarray(offsets, dtype=np.float32))
    imgs = images.reshape(PAIRS_TOTAL, C, H, W)
    offp = offsets.reshape(4, 8, 2, H, W).reshape(PAIRS_TOTAL, 2, H, W)

    nc = build_nc()
    in_maps = []
    for core in range(N_CORES):
        sl = slice(core * PAIRS, (core + 1) * PAIRS)
        in_maps.append(make_in_map(imgs[sl], offp[sl]))
    trace = bool(os.environ.get("DK_TRACE"))
    res = run_bass_kernel_spmd(nc, in_maps, list(range(N_CORES)), trace=trace)
    if trace:
        LAST_EXEC_TIME_NS = res.exec_time_ns
        if res.instructions_and_trace:
            print("trace path:", res.instructions_and_trace[1])
    full = np.empty((PAIRS_TOTAL, C, H, W), np.float32)
    for i in range(N_CORES):
        od = np.asarray(res.results[i]["out"]).astype(np.float32)       # (4,H,W_D,C)
        op = np.asarray(res.results[i]["out_p"]).astype(np.float32)     # (128, PP)
        sl = slice(i * PAIRS, (i + 1) * PAIRS)
        full[sl, :, :, :W_D] = np.transpose(od, (0, 3, 1, 2))
        opp = op.reshape(PAIRS, C, W_P, H)                              # i = w'*128+h
        full[sl, :, :, W_D:] = np.transpose(opp, (0, 1, 3, 2))
    return np.ascontiguousarray(full.reshape(4, 8, C, H, W)).astype(np.float32)



# revision 6
# speedup vs baseline: 1.2183x; 1.2183x over previous
"""Deformable bilinear sampling TRN2 kernel, v2: full DMA-gather design.

Patch rows are (c, k)-interleaved (c-major, 4 corners minor) so the whole
4-corner weighted product is ONE DVE tensor_tensor at 2x (the per-corner
weight tile broadcasts over the middle c dim; last dim k stays packed),
followed by a 2x pair-add over k-halves and a strided final pair-add split
between Pool and DVE. Pool runs the 16 full-width gathers (~3.4us each,
byte-bound); ACT runs the index wrap dance; SP streams outputs.
"""

import numpy as np

import concourse.bacc as bacc
import concourse.bass as bass
import concourse.mybir as mybir
from concourse.library_config import mlp

PAIRS = 4
H = W = 128
C = 32
K = 4
PAD = 8
HP = 144
NROWS = HP * HP
CH = 4                    # chunks per pair
WCH = W // CH             # 32 w-cols per chunk
NIDX = H * WCH            # 4096 indices per chunk
NCHUNK = PAIRS * CH       # 16

F32 = mybir.dt.float32
BF16 = mybir.dt.bfloat16
I16 = mybir.dt.int16
OP = mybir.AluOpType
TWO23 = 12582912.0

CHUNKS = [(c // 4, 32 * (c % 4), 32) for c in range(15)] + [(3, 96, 16), (3, 112, 16)]
NC_ = len(CHUNKS)

NG = 4                    # gather buffers
NP = 2                    # product buffers
NT = 3                    # T2 buffers
NR = 3                    # result buffers


def build_nc(final_split=None):
    # which engine does the final pair-add per chunk: 'g' Pool, 'v' DVE
    if final_split is None:
        final_split = ["g"] * NC_
        for i in (2, 5, 8, 11, 14):
            final_split[i] = "v"
    nc = bacc.Bacc("TRN2")
    patches = nc.declare_dram_parameter("patches", [PAIRS, NROWS, 128], BF16, isOutput=False)
    offn = nc.declare_dram_parameter("offn", [PAIRS, 2, H, W], F32, isOutput=False)
    basen = nc.declare_dram_parameter("basen", [H, W], F32, isOutput=False)
    out = nc.declare_dram_parameter("out", [PAIRS, H, W, C], BF16, isOutput=True)

    from contextlib import ExitStack

    with ExitStack() as stack:
        ec = stack.enter_context
        block = ec(nc.Block())
        Gb = [ec(nc.sbuf_tensor(f"G{i}", [128, WCH, C, K], BF16)) for i in range(NG)]
        Pb = [ec(nc.sbuf_tensor(f"P{i}", [128, WCH, C, K], BF16)) for i in range(NP)]
        T2b = [ec(nc.sbuf_tensor(f"T2_{i}", [128, WCH, C, 2], BF16)) for i in range(NT)]
        Rb = [ec(nc.sbuf_tensor(f"R{i}", [128, WCH, C], BF16)) for i in range(NR)]
        onb = ec(nc.sbuf_tensor("onb", [128, 2 * PAIRS, W], F32))   # (pair, ch) interleaved p*2+ch, pair-major
        sy2 = ec(nc.sbuf_tensor("sy2", [128, 2 * PAIRS, W], F32))
        sf = ec(nc.sbuf_tensor("sf", [128, 2 * PAIRS, W], F32))
        sg = ec(nc.sbuf_tensor("sg", [128, 2 * PAIRS, W], F32))
        tD = ec(nc.sbuf_tensor("tD", [128, PAIRS, W], F32))
        dnat = ec(nc.sbuf_tensor("dnat", [128, PAIRS, W], I16))
        wt4 = ec(nc.sbuf_tensor("wt4", [128, PAIRS, W, K], BF16))
        bnat = ec(nc.sbuf_tensor("bnat", [128, W], F32))
        db = [ec(nc.sbuf_tensor(f"d{p}", [128, H * W // 16], I16)) for p in range(PAIRS)]

        s_inx = [ec(nc.semaphore(f"s_inx{p}")) for p in range(PAIRS)]
        s_inb = ec(nc.semaphore("s_inb"))
        s_dn = [ec(nc.semaphore(f"s_dn{p}")) for p in range(PAIRS)]
        s_wt = ec(nc.semaphore("s_wt"))
        s_dw = [ec(nc.semaphore(f"s_dw{p}")) for p in range(PAIRS)]
        s_g = [ec(nc.semaphore(f"s_g{i}")) for i in range(NC_)]
        s_mul = [ec(nc.semaphore(f"s_mul{i}")) for i in range(NC_)]
        s_t2 = [ec(nc.semaphore(f"s_t2_{i}")) for i in range(NC_)]
        s_fin = [ec(nc.semaphore(f"s_fin{i}")) for i in range(NC_)]
        s_out = [ec(nc.semaphore(f"s_out{i}")) for i in range(NC_)]
        s_cv = ec(nc.semaphore("s_cv"))
        s_cg = ec(nc.semaphore("s_cg"))

        class Chain:
            """Serialize same-engine ops through one counting semaphore."""

            def __init__(self, eng, sem):
                self.eng, self.sem, self.n = eng, sem, 0

            def run(self, thunk, waits=(), final=None):
                if self.n:
                    self.eng.wait_ge(self.sem, self.n)
                for sem, val in waits:
                    self.eng.wait_ge(sem, val)
                inst = thunk()
                if final is None:
                    inst.then_inc(self.sem, 1)
                    self.n += 1
                else:
                    inst.then_inc(*final)
                return inst

        @block.sync
        def _(sync: bass.BassEngine):
            sync.dma_start(bnat[:, :], basen[:, :]).then_inc(s_inb, 16)
            for p in range(PAIRS):
                sync.dma_start(
                    onb[:, 2 * p:2 * p + 2, :],
                    offn[p, :, :, :].transpose([1, 0, 2]),
                ).then_inc(s_inx[p], 16)
            for cidx, (p, w0, nw) in enumerate(CHUNKS):
                sync.wait_ge(s_fin[cidx], 1)
                dst = out[p, :, w0:w0 + nw, :]
                sync.dma_start(dst, Rb[cidx % NR][:, 0:nw, :]).then_inc(s_out[cidx], 16)

        @block.vector
        def _(v: bass.BassEngine):
            ch = Chain(v, s_cv)
            r = ch.run

            def floor_anchor(sl, tsl, dn_batched):
                """Floor + anchors over onb channel slice sl; frac sub deferred."""
                onf = onb[:, sl, :]
                r(lambda: v.tensor_scalar(sy2[:, sl, :], onf, TWO23, -TWO23, OP.add, OP.add))
                r(lambda: v.tensor_tensor(sf[:, sl, :], sy2[:, sl, :], onf, OP.is_gt))
                r(lambda: v.tensor_sub(sy2[:, sl, :], sy2[:, sl, :], sf[:, sl, :]))
                npair = (sl.stop - sl.start) // 2
                hs = slice(sl.start, sl.stop, 2)
                ws = slice(sl.start + 1, sl.stop, 2)
                r(lambda: v.scalar_tensor_tensor(
                    tD[:, tsl, :], sy2[:, hs, :], float(HP), sy2[:, ws, :], OP.mult, OP.add),
                  waits=[(s_inb, 16)])
                r(lambda: v.tensor_tensor(
                    tD[:, tsl, :], tD[:, tsl, :],
                    bnat[:, :].unsqueeze(1).broadcast_to([128, npair, W]), OP.add))
                if dn_batched:
                    r(lambda: v.tensor_copy(dnat[:, tsl, :], tD[:, tsl, :]),
                      final=(s_dn[1], 1))
                else:
                    for p in range(tsl.start, tsl.stop):
                        r(lambda p=p: v.tensor_copy(dnat[:, p, :], tD[:, p, :]),
                          final=(s_dn[p], 1))
                r(lambda: v.tensor_sub(sf[:, sl, :], onf, sy2[:, sl, :]))

            # pair 0 fast path unblocks ACT wrap + first gathers ASAP
            v.wait_ge(s_inx[0], 16)
            floor_anchor(slice(0, 2), slice(0, 1), False)
            for p in range(1, PAIRS):
                v.wait_ge(s_inx[p], 16)
            floor_anchor(slice(2, 8), slice(1, 4), True)
            # weights: wt4[:, p, w, k]; k = 2*dh + dw
            r(lambda: v.tensor_scalar(sg[:, :, :], sf[:, :, :], -1.0, 1.0, OP.mult, OP.add))
            hsel = {0: sg, 1: sf}
            for kk in range(K):
                a, b = divmod(kk, 2)
                fin = (s_wt, 1) if kk == K - 1 else None
                r(lambda a=a, b=b, kk=kk: v.tensor_tensor(
                    wt4[:, :, :, kk], hsel[a][:, 0::2, :], hsel[b][:, 1::2, :], OP.mult),
                  final=fin)

            for cidx, (p, w0, nw) in enumerate(CHUNKS):
                P = Pb[cidx % NP]
                T2 = T2b[cidx % NT]
                wv = wt4[:, p, w0:w0 + nw, :].unsqueeze(2).broadcast_to(
                    [128, nw, C, K])
                waits = [(s_g[cidx], 16)]
                if cidx == 0:
                    waits.append((s_wt, 1))
                if cidx >= NP:
                    waits.append((s_t2[cidx - NP], 1))   # P buf reuse: addk done
                r(lambda P=P, wv=wv, G=Gb[cidx % NG], nw=nw: v.tensor_tensor(
                    P[:, 0:nw, :, :], G[:, 0:nw, :, :], wv, OP.mult),
                  waits=waits, final=(s_mul[cidx], 1))
                v.wait_ge(s_mul[cidx], 1)
                waits = []
                if cidx >= NT:
                    waits.append((s_fin[cidx - NT], 1))  # T2 buf reuse
                r(lambda P=P, T2=T2, nw=nw: v.tensor_tensor(
                    T2[:, 0:nw, :, :], P[:, 0:nw, :, 0:2], P[:, 0:nw, :, 2:4], OP.add),
                  waits=waits, final=(s_t2[cidx], 1))
                if final_split[cidx] == "v":
                    v.wait_ge(s_t2[cidx], 1)
                    waits = []
                    if cidx >= NR:
                        waits.append((s_out[cidx - NR], 16))
                    r(lambda T2=T2, R=Rb[cidx % NR], nw=nw: v.tensor_tensor(
                        R[:, 0:nw, :], T2[:, 0:nw, :, 0], T2[:, 0:nw, :, 1], OP.add),
                      waits=waits, final=(s_fin[cidx], 1))

        @block.scalar
        def _(act: bass.BassEngine):
            for p in range(PAIRS):
                act.memzero(db[p][:, :].bitcast(BF16)).then_inc(s_cg, 1)
            act.wait_ge(s_cg, 4)
            for p in range(PAIRS):
                act.wait_ge(s_dn[min(p, 1)], 1)
                dwrap = db[p][:, :].rearrange("q (w k) -> q w k", k=8)
                with nc.allow_non_contiguous_dma(reason="idx-wrap strided dst"):
                    for k in (1, 3, 5, 7):
                        act.dma_start(dwrap[0:16, :, k],
                                      dnat[16 * k:16 * (k + 1), p, :]).then_inc(s_dw[p], 16)
                for k in range(0, 8, 2):
                    act.copy(dwrap[0:16, :, k],
                             dnat[16 * k:16 * (k + 1), p, :]).then_inc(s_dw[p], 1)
                act.wait_ge(s_dw[p], 4 + 64)
                act.dma_start(db[p][16:32, :],
                              db[p][0:16, :]).then_inc(s_dw[p], 16)

        @block.gpsimd
        def _(g: bass.BassGpSimd):
            chg = Chain(g, s_cg)
            g.load_library(mlp)
            n_ms = 0
            pool_finals = []
            emitted = 0

            def emit_final(cidx):
                waits = [(s_t2[cidx], 1)]
                if cidx >= NR:
                    waits.append((s_out[cidx - NR], 16))
                T2 = T2b[cidx % NT]
                nw = CHUNKS[cidx][2]
                chg.run(lambda T2=T2, R=Rb[cidx % NR], nw=nw: g.tensor_tensor(
                    R[:, 0:nw, :], T2[:, 0:nw, :, 0], T2[:, 0:nw, :, 1], OP.add),
                    waits=waits, final=(s_fin[cidx], 1))

            for cidx, (p, w0, nw) in enumerate(CHUNKS):
                waits = [(s_dw[p], 4 + 64 + 16)]
                if cidx >= NG:
                    waits.append((s_mul[cidx - NG], 1))
                for sem, val in waits:
                    g.wait_ge(sem, val)
                nidx = H * nw
                g.dma_gather(
                    Gb[cidx % NG][:, 0:nw, :, :].rearrange("q w c k -> q w (c k)"),
                    patches[p, :, :],
                    db[p][:, w0 * 8:(w0 + nw) * 8],
                    nidx,
                    nidx,
                    128,
                    single_packet=False,
                ).then_inc(s_g[cidx], 16)
                # weave pool finals behind the gather stream
                while (emitted < len(pool_finals) and
                       pool_finals[emitted] <= cidx - 2):
                    emit_final(pool_finals[emitted])
                    emitted += 1
                if final_split[cidx] == "g":
                    pool_finals.append(cidx)
            while emitted < len(pool_finals):
                emit_final(pool_finals[emitted])
                emitted += 1

    nc.compile()
    return nc


# ---------------- host-side helpers ----------------

def build_patches_all(imgs_pairs):
    """(npair, C, H, W) f32 -> (npair, NROWS, 128) bf16, rows (c, k)."""
    import ml_dtypes

    npair = imgs_pairs.shape[0]
    hw_c = np.ascontiguousarray(np.transpose(imgs_pairs, (0, 2, 3, 1)))  # (n, H, W, C)
    padded = np.zeros((npair, HP + 1, HP + 1, C), np.float32)
    padded[:, PAD:PAD + H, PAD:PAD + W] = hw_c
    P = np.empty((npair, HP, HP, C, K), np.float32)
    P[:, :, :, :, 0] = padded[:, 0:HP, 0:HP]
    P[:, :, :, :, 1] = padded[:, 0:HP, 1:HP + 1]
    P[:, :, :, :, 2] = padded[:, 1:HP + 1, 0:HP]
    P[:, :, :, :, 3] = padded[:, 1:HP + 1, 1:HP + 1]
    return P.reshape(npair, NROWS, 128).astype(ml_dtypes.bfloat16)


def base_natural():
    h = np.arange(H).reshape(H, 1)
    w = np.arange(W).reshape(1, W)
    return ((h + PAD) * HP + (w + PAD)).astype(np.float32)


def make_in_map(imgs_pairs, offp):
    return {
        "patches": build_patches_all(imgs_pairs),
        "offn": np.ascontiguousarray(offp),
        "basen": base_natural(),
    }


# ---------------- public entry point ----------------

N_CORES = 8
PAIRS_TOTAL = 32

LAST_EXEC_TIME_NS = None


def kernel(images, offsets):
    import os
    global LAST_EXEC_TIME_NS
    from concourse.bass_utils import run_bass_kernel_spmd

    images = np.ascontiguousarray(np.asarray(images, dtype=np.float32))
    offsets = np.ascontiguousarray(np.asarray(offsets, dtype=np.float32))
    imgs = images.reshape(PAIRS_TOTAL, C, H, W)
    offp = offsets.reshape(4, 8, 2, H, W).reshape(PAIRS_TOTAL, 2, H, W)

    nc = build_nc()
    in_maps = []
    for core in range(N_CORES):
        sl = slice(core * PAIRS, (core + 1) * PAIRS)
        in_maps.append(make_in_map(imgs[sl], offp[sl]))
    trace = bool(os.environ.get("DK_TRACE"))
    res = run_bass_kernel_spmd(nc, in_maps, list(range(N_CORES)), trace=trace)
    if trace:
        LAST_EXEC_TIME_NS = res.exec_time_ns
        if res.instructions_and_trace:
            print("trace path:", res.instructions_and_trace[1])
    full = np.empty((PAIRS_TOTAL, C, H, W), np.float32)
    for i in range(N_CORES):
        od = np.asarray(res.results[i]["out"]).astype(np.float32)   # (4, H, W, C)
        sl = slice(i * PAIRS, (i + 1) * PAIRS)
        full[sl] = np.transpose(od, (0, 3, 1, 2))
    return np.ascontiguousarray(full.reshape(4, 8, C, H, W)).astype(np.float32)


# revision 9
# speedup vs baseline: 1.2386x; 1.0166x over previous
"""Deformable bilinear sampling TRN2 kernel, v2: full DMA-gather design.

Patch rows are (c, k)-interleaved (c-major, 4 corners minor) so the whole
4-corner weighted product is ONE DVE tensor_tensor at 2x (the per-corner
weight tile broadcasts over the middle c dim; last dim k stays packed),
followed by a 2x pair-add over k-halves and a strided final pair-add split
between Pool and DVE. Pool runs the 16 full-width gathers (~3.4us each,
byte-bound); ACT runs the index wrap dance; SP streams outputs.
"""

import numpy as np

import concourse.bacc as bacc
import concourse.bass as bass
import concourse.mybir as mybir
from concourse.library_config import mlp

PAIRS = 4
H = W = 128
C = 32
K = 4
PAD = 8
HP = 144
NROWS = HP * HP
CH = 4                    # chunks per pair
WCH = W // CH             # 32 w-cols per chunk
NIDX = H * WCH            # 4096 indices per chunk
NCHUNK = PAIRS * CH       # 16

F32 = mybir.dt.float32
BF16 = mybir.dt.bfloat16
I16 = mybir.dt.int16
OP = mybir.AluOpType
TWO23 = 12582912.0

CHUNKS = [(c // 4, 32 * (c % 4), 32) for c in range(15)] + [(3, 96, 16), (3, 112, 16)]
NC_ = len(CHUNKS)

NG = 4                    # gather buffers
NP = 2                    # product buffers
NT = 3                    # T2 buffers
NR = 3                    # result buffers


def build_nc(final_split=None):
    # which engine does the final pair-add per chunk: 'g' Pool, 'v' DVE
    if final_split is None:
        final_split = ["g"] * NC_
        for i in (2, 5, 8, 11, 14):
            final_split[i] = "v"
    nc = bacc.Bacc("TRN2")
    patches = nc.declare_dram_parameter("patches", [PAIRS, NROWS, 128], BF16, isOutput=False)
    offn = nc.declare_dram_parameter("offn", [PAIRS, 2, H, W], F32, isOutput=False)
    basen = nc.declare_dram_parameter("basen", [H, W], F32, isOutput=False)
    out = nc.declare_dram_parameter("out", [PAIRS, H, W, C], BF16, isOutput=True)

    from contextlib import ExitStack

    with ExitStack() as stack:
        ec = stack.enter_context
        block = ec(nc.Block())
        Gb = [ec(nc.sbuf_tensor(f"G{i}", [128, WCH, C, K], BF16)) for i in range(NG)]
        Pb = [ec(nc.sbuf_tensor(f"P{i}", [128, WCH, C, K], BF16)) for i in range(NP)]
        T2b = [ec(nc.sbuf_tensor(f"T2_{i}", [128, WCH, C, 2], BF16)) for i in range(NT)]
        Rb = [ec(nc.sbuf_tensor(f"R{i}", [128, WCH, C], BF16)) for i in range(NR)]
        onb = ec(nc.sbuf_tensor("onb", [128, 2 * PAIRS, W], F32))   # (pair, ch) interleaved p*2+ch, pair-major
        sy2 = ec(nc.sbuf_tensor("sy2", [128, 2 * PAIRS, W], F32))
        sf = ec(nc.sbuf_tensor("sf", [128, 2 * PAIRS, W], F32))
        sg = ec(nc.sbuf_tensor("sg", [128, 2 * PAIRS, W], F32))
        tD = ec(nc.sbuf_tensor("tD", [128, PAIRS, W], F32))
        dnat = ec(nc.sbuf_tensor("dnat", [128, PAIRS, W], I16))
        wt4 = ec(nc.sbuf_tensor("wt4", [128, PAIRS, W, K], BF16))
        bnat = ec(nc.sbuf_tensor("bnat", [128, W], F32))
        db = [ec(nc.sbuf_tensor(f"d{p}", [128, H * W // 16], I16)) for p in range(PAIRS)]

        s_inx = [ec(nc.semaphore(f"s_inx{p}")) for p in range(PAIRS)]
        s_inb = ec(nc.semaphore("s_inb"))
        s_dn = [ec(nc.semaphore(f"s_dn{p}")) for p in range(PAIRS)]
        s_wt = ec(nc.semaphore("s_wt"))
        s_dw = [ec(nc.semaphore(f"s_dw{p}")) for p in range(PAIRS)]
        s_g = [ec(nc.semaphore(f"s_g{i}")) for i in range(NC_)]
        s_mul = [ec(nc.semaphore(f"s_mul{i}")) for i in range(NC_)]
        s_t2 = [ec(nc.semaphore(f"s_t2_{i}")) for i in range(NC_)]
        s_fin = [ec(nc.semaphore(f"s_fin{i}")) for i in range(NC_)]
        s_out = [ec(nc.semaphore(f"s_out{i}")) for i in range(NC_)]
        s_cv = ec(nc.semaphore("s_cv"))
        s_cg = ec(nc.semaphore("s_cg"))

        class Chain:
            """Serialize same-engine ops through one counting semaphore."""

            def __init__(self, eng, sem):
                self.eng, self.sem, self.n = eng, sem, 0

            def run(self, thunk, waits=(), final=None):
                if self.n:
                    self.eng.wait_ge(self.sem, self.n)
                for sem, val in waits:
                    self.eng.wait_ge(sem, val)
                inst = thunk()
                if final is None:
                    inst.then_inc(self.sem, 1)
                    self.n += 1
                else:
                    inst.then_inc(*final)
                return inst

        @block.sync
        def _(sync: bass.BassEngine):
            sync.dma_start(bnat[:, :], basen[:, :]).then_inc(s_inb, 16)
            for p in range(PAIRS):
                sync.dma_start(
                    onb[:, 2 * p:2 * p + 2, :],
                    offn[p, :, :, :].transpose([1, 0, 2]),
                ).then_inc(s_inx[p], 16)
            for cidx, (p, w0, nw) in enumerate(CHUNKS):
                sync.wait_ge(s_fin[cidx], 1)
                dst = out[p, :, w0:w0 + nw, :]
                sync.dma_start(dst, Rb[cidx % NR][:, 0:nw, :]).then_inc(s_out[cidx], 16)

        @block.vector
        def _(v: bass.BassEngine):
            ch = Chain(v, s_cv)
            r = ch.run

            def floor_anchor(sl, tsl, dn_batched):
                """Floor + anchors over onb channel slice sl; frac sub deferred."""
                onf = onb[:, sl, :]
                r(lambda: v.tensor_scalar(sy2[:, sl, :], onf, TWO23, -TWO23, OP.add, OP.add))
                r(lambda: v.tensor_tensor(sf[:, sl, :], sy2[:, sl, :], onf, OP.is_gt))
                r(lambda: v.tensor_sub(sy2[:, sl, :], sy2[:, sl, :], sf[:, sl, :]))
                npair = (sl.stop - sl.start) // 2
                hs = slice(sl.start, sl.stop, 2)
                ws = slice(sl.start + 1, sl.stop, 2)
                r(lambda: v.scalar_tensor_tensor(
                    tD[:, tsl, :], sy2[:, hs, :], float(HP), sy2[:, ws, :], OP.mult, OP.add),
                  waits=[(s_inb, 16)])
                r(lambda: v.tensor_tensor(
                    tD[:, tsl, :], tD[:, tsl, :],
                    bnat[:, :].unsqueeze(1).broadcast_to([128, npair, W]), OP.add))
                if dn_batched:
                    r(lambda: v.tensor_copy(dnat[:, tsl, :], tD[:, tsl, :]),
                      final=(s_dn[1], 1))
                else:
                    for p in range(tsl.start, tsl.stop):
                        r(lambda p=p: v.tensor_copy(dnat[:, p, :], tD[:, p, :]),
                          final=(s_dn[p], 1))
                r(lambda: v.tensor_sub(sf[:, sl, :], onf, sy2[:, sl, :]))

            # pair 0 fast path unblocks ACT wrap + first gathers ASAP
            v.wait_ge(s_inx[0], 16)
            floor_anchor(slice(0, 2), slice(0, 1), False)
            for p in range(1, PAIRS):
                v.wait_ge(s_inx[p], 16)
            floor_anchor(slice(2, 8), slice(1, 4), True)
            # weights: wt4[:, p, w, k]; k = 2*dh + dw
            r(lambda: v.tensor_scalar(sg[:, :, :], sf[:, :, :], -1.0, 1.0, OP.mult, OP.add))
            hsel = {0: sg, 1: sf}
            for kk in range(K):
                a, b = divmod(kk, 2)
                fin = (s_wt, 1) if kk == K - 1 else None
                r(lambda a=a, b=b, kk=kk: v.tensor_tensor(
                    wt4[:, :, :, kk], hsel[a][:, 0::2, :], hsel[b][:, 1::2, :], OP.mult),
                  final=fin)

            for cidx, (p, w0, nw) in enumerate(CHUNKS):
                P = Pb[cidx % NP]
                T2 = T2b[cidx % NT]
                wv = wt4[:, p, w0:w0 + nw, :].unsqueeze(2).broadcast_to(
                    [128, nw, C, K])
                waits = [(s_g[cidx], 16)]
                if cidx == 0:
                    waits.append((s_wt, 1))
                if cidx >= NP:
                    waits.append((s_t2[cidx - NP], 1))   # P buf reuse: addk done
                r(lambda P=P, wv=wv, G=Gb[cidx % NG], nw=nw: v.tensor_tensor(
                    P[:, 0:nw, :, :], G[:, 0:nw, :, :], wv, OP.mult),
                  waits=waits, final=(s_mul[cidx], 1))
                v.wait_ge(s_mul[cidx], 1)
                waits = []
                if cidx >= NT:
                    waits.append((s_fin[cidx - NT], 1))  # T2 buf reuse
                r(lambda P=P, T2=T2, nw=nw: v.tensor_tensor(
                    T2[:, 0:nw, :, :], P[:, 0:nw, :, 0:2], P[:, 0:nw, :, 2:4], OP.add),
                  waits=waits, final=(s_t2[cidx], 1))
                if final_split[cidx] == "v":
                    v.wait_ge(s_t2[cidx], 1)
                    waits = []
                    if cidx >= NR:
                        waits.append((s_out[cidx - NR], 16))
                    r(lambda T2=T2, R=Rb[cidx % NR], nw=nw: v.tensor_tensor(
                        R[:, 0:nw, :], T2[:, 0:nw, :, 0], T2[:, 0:nw, :, 1], OP.add),
                      waits=waits, final=(s_fin[cidx], 1))

        @block.scalar
        def _(act: bass.BassEngine):
            for p in range(PAIRS):
                act.memzero(db[p][:, :].bitcast(BF16)).then_inc(s_cg, 1)
            act.wait_ge(s_cg, 4)
            for p in range(PAIRS):
                act.wait_ge(s_dn[min(p, 1)], 1)
                dwrap = db[p][:, :].rearrange("q (w k) -> q w k", k=8)
                for k in range(0, 8, 2):
                    act.copy(dwrap[0:16, :, k],
                             dnat[16 * k:16 * (k + 1), p, :]).then_inc(s_dw[p], 1)
                act.wait_ge(s_dw[p], 4)
                with nc.allow_non_contiguous_dma(reason="idx-wrap strided dst"):
                    for k in (1, 3, 5, 7):
                        act.dma_start(dwrap[0:16, :, k],
                                      dnat[16 * k:16 * (k + 1), p, :]).then_inc(s_dw[p], 16)
                act.wait_ge(s_dw[p], 4 + 64)
                act.dma_start(db[p][16:32, :],
                              db[p][0:16, :]).then_inc(s_dw[p], 16)

        @block.gpsimd
        def _(g: bass.BassGpSimd):
            chg = Chain(g, s_cg)
            g.load_library(mlp)
            n_ms = 0
            pool_finals = []
            emitted = 0

            def emit_final(cidx):
                waits = [(s_t2[cidx], 1)]
                if cidx >= NR:
                    waits.append((s_out[cidx - NR], 16))
                T2 = T2b[cidx % NT]
                nw = CHUNKS[cidx][2]
                chg.run(lambda T2=T2, R=Rb[cidx % NR], nw=nw: g.tensor_tensor(
                    R[:, 0:nw, :], T2[:, 0:nw, :, 0], T2[:, 0:nw, :, 1], OP.add),
                    waits=waits, final=(s_fin[cidx], 1))

            for cidx, (p, w0, nw) in enumerate(CHUNKS):
                waits = [(s_dw[p], 4 + 64 + 16)]
                if cidx >= NG:
                    waits.append((s_mul[cidx - NG], 1))
                for sem, val in waits:
                    g.wait_ge(sem, val)
                nidx = H * nw
                g.dma_gather(
                    Gb[cidx % NG][:, 0:nw, :, :].rearrange("q w c k -> q w (c k)"),
                    patches[p, :, :],
                    db[p][:, w0 * 8:(w0 + nw) * 8],
                    nidx,
                    nidx,
                    128,
                    single_packet=False,
                ).then_inc(s_g[cidx], 16)
                # weave pool finals behind the gather stream
                while (emitted < len(pool_finals) and
                       pool_finals[emitted] <= cidx - 2):
                    emit_final(pool_finals[emitted])
                    emitted += 1
                if final_split[cidx] == "g":
                    pool_finals.append(cidx)
            while emitted < len(pool_finals):
                emit_final(pool_finals[emitted])
                emitted += 1

    nc.compile()
    return nc


# ---------------- host-side helpers ----------------

def build_patches_all(imgs_pairs):
    """(npair, C, H, W) f32 -> (npair, NROWS, 128) bf16, rows (c, k)."""
    import ml_dtypes

    npair = imgs_pairs.shape[0]
    hw_c = np.ascontiguousarray(np.transpose(imgs_pairs, (0, 2, 3, 1)))  # (n, H, W, C)
    padded = np.zeros((npair, HP + 1, HP + 1, C), np.float32)
    padded[:, PAD:PAD + H, PAD:PAD + W] = hw_c
    P = np.empty((npair, HP, HP, C, K), np.float32)
    P[:, :, :, :, 0] = padded[:, 0:HP, 0:HP]
    P[:, :, :, :, 1] = padded[:, 0:HP, 1:HP + 1]
    P[:, :, :, :, 2] = padded[:, 1:HP + 1, 0:HP]
    P[:, :, :, :, 3] = padded[:, 1:HP + 1, 1:HP + 1]
    return P.reshape(npair, NROWS, 128).astype(ml_dtypes.bfloat16)


def base_natural():
    h = np.arange(H).reshape(H, 1)
    w = np.arange(W).reshape(1, W)
    return ((h + PAD) * HP + (w + PAD)).astype(np.float32)


def make_in_map(imgs_pairs, offp):
    return {
        "patches": build_patches_all(imgs_pairs),
        "offn": np.ascontiguousarray(offp),
        "basen": base_natural(),
    }


# ---------------- public entry point ----------------

N_CORES = 8
PAIRS_TOTAL = 32

LAST_EXEC_TIME_NS = None


def kernel(images, offsets):
    import os
    global LAST_EXEC_TIME_NS
    from concourse.bass_utils import run_bass_kernel_spmd

    images = np.ascontiguousarray(np.asarray(images, dtype=np.float32))
    offsets = np.ascontiguousarray(np.asarray(offsets, dtype=np.float32))
    imgs = images.reshape(PAIRS_TOTAL, C, H, W)
    offp = offsets.reshape(4, 8, 2, H, W).reshape(PAIRS_TOTAL, 2, H, W)

    nc = build_nc()
    in_maps = []
    for core in range(N_CORES):
        sl = slice(core * PAIRS, (core + 1) * PAIRS)
        in_maps.append(make_in_map(imgs[sl], offp[sl]))
    trace = bool(os.environ.get("DK_TRACE"))
    res = run_bass_kernel_spmd(nc, in_maps, list(range(N_CORES)), trace=trace)
    if trace:
        LAST_EXEC_TIME_NS = res.exec_time_ns
        if res.instructions_and_trace:
            print("trace path:", res.instructions_and_trace[1])
    full = np.empty((PAIRS_TOTAL, C, H, W), np.float32)
    for i in range(N_CORES):
        od = np.asarray(res.results[i]["out"]).astype(np.float32)   # (4, H, W, C)
        sl = slice(i * PAIRS, (i + 1) * PAIRS)
        full[sl] = np.transpose(od, (0, 3, 1, 2))
    return np.ascontiguousarray(full.reshape(4, 8, C, H, W)).astype(np.float32)


# revision 10
# speedup vs baseline: 1.2538x; 1.0123x over previous
"""Deformable bilinear sampling TRN2 kernel, v2: full DMA-gather design.

Patch rows are (c, k)-interleaved (c-major, 4 corners minor) so the whole
4-corner weighted product is ONE DVE tensor_tensor at 2x (the per-corner
weight tile broadcasts over the middle c dim; last dim k stays packed),
followed by a 2x pair-add over k-halves and a strided final pair-add split
between Pool and DVE. Pool runs the 16 full-width gathers (~3.4us each,
byte-bound); ACT runs the index wrap dance; SP streams outputs.
"""

import numpy as np

import concourse.bacc as bacc
import concourse.bass as bass
import concourse.mybir as mybir
from concourse.library_config import mlp

PAIRS = 4
H = W = 128
C = 32
K = 4
PAD = 8
HP = 144
NROWS = HP * HP
CH = 4                    # chunks per pair
WCH = W // CH             # 32 w-cols per chunk
NIDX = H * WCH            # 4096 indices per chunk
NCHUNK = PAIRS * CH       # 16

F32 = mybir.dt.float32
BF16 = mybir.dt.bfloat16
I16 = mybir.dt.int16
OP = mybir.AluOpType
TWO23 = 12582912.0

CHUNKS = [(c // 4, 32 * (c % 4), 32) for c in range(15)] + [(3, 96, 16), (3, 112, 16)]
NC_ = len(CHUNKS)

NG = 4                    # gather buffers
NP = 2                    # product buffers
NT = 3                    # T2 buffers
NR = 3                    # result buffers


def build_nc(final_split=None):
    # which engine does the final pair-add per chunk: 'g' Pool, 'v' DVE
    if final_split is None:
        final_split = ["g"] * NC_
        for i in (1, 4, 7, 10, 13):
            final_split[i] = "v"
    nc = bacc.Bacc("TRN2")
    patches = nc.declare_dram_parameter("patches", [PAIRS, NROWS, 128], BF16, isOutput=False)
    offn = nc.declare_dram_parameter("offn", [PAIRS, 2, H, W], F32, isOutput=False)
    basen = nc.declare_dram_parameter("basen", [H, W], F32, isOutput=False)
    out = nc.declare_dram_parameter("out", [PAIRS, H, W, C], BF16, isOutput=True)

    from contextlib import ExitStack

    with ExitStack() as stack:
        ec = stack.enter_context
        block = ec(nc.Block())
        Gb = [ec(nc.sbuf_tensor(f"G{i}", [128, WCH, C, K], BF16)) for i in range(NG)]
        Pb = [ec(nc.sbuf_tensor(f"P{i}", [128, WCH, C, K], BF16)) for i in range(NP)]
        T2b = [ec(nc.sbuf_tensor(f"T2_{i}", [128, WCH, C, 2], BF16)) for i in range(NT)]
        Rb = [ec(nc.sbuf_tensor(f"R{i}", [128, WCH, C], BF16)) for i in range(NR)]
        onb = ec(nc.sbuf_tensor("onb", [128, 2 * PAIRS, W], F32))   # (pair, ch) interleaved p*2+ch, pair-major
        sy2 = ec(nc.sbuf_tensor("sy2", [128, 2 * PAIRS, W], F32))
        sf = ec(nc.sbuf_tensor("sf", [128, 2 * PAIRS, W], F32))
        sg = ec(nc.sbuf_tensor("sg", [128, 2 * PAIRS, W], F32))
        tD = ec(nc.sbuf_tensor("tD", [128, PAIRS, W], F32))
        dnat = ec(nc.sbuf_tensor("dnat", [128, PAIRS, W], I16))
        wt4 = ec(nc.sbuf_tensor("wt4", [128, PAIRS, W, K], BF16))
        bnat = ec(nc.sbuf_tensor("bnat", [128, W], F32))
        db = [ec(nc.sbuf_tensor(f"d{p}", [128, H * W // 16], I16)) for p in range(PAIRS)]

        s_inx = [ec(nc.semaphore(f"s_inx{p}")) for p in range(PAIRS)]
        s_inb = ec(nc.semaphore("s_inb"))
        s_dn = [ec(nc.semaphore(f"s_dn{p}")) for p in range(PAIRS)]
        s_wt = ec(nc.semaphore("s_wt"))
        s_dw = [ec(nc.semaphore(f"s_dw{p}")) for p in range(PAIRS)]
        s_g = [ec(nc.semaphore(f"s_g{i}")) for i in range(NC_)]
        s_mul = [ec(nc.semaphore(f"s_mul{i}")) for i in range(NC_)]
        s_t2 = [ec(nc.semaphore(f"s_t2_{i}")) for i in range(NC_)]
        s_fin = [ec(nc.semaphore(f"s_fin{i}")) for i in range(NC_)]
        s_out = [ec(nc.semaphore(f"s_out{i}")) for i in range(NC_)]
        s_cv = ec(nc.semaphore("s_cv"))
        s_cg = ec(nc.semaphore("s_cg"))

        class Chain:
            """Serialize same-engine ops through one counting semaphore."""

            def __init__(self, eng, sem):
                self.eng, self.sem, self.n = eng, sem, 0

            def run(self, thunk, waits=(), final=None):
                if self.n:
                    self.eng.wait_ge(self.sem, self.n)
                for sem, val in waits:
                    self.eng.wait_ge(sem, val)
                inst = thunk()
                if final is None:
                    inst.then_inc(self.sem, 1)
                    self.n += 1
                else:
                    inst.then_inc(*final)
                return inst

        @block.sync
        def _(sync: bass.BassEngine):
            sync.dma_start(bnat[:, :], basen[:, :]).then_inc(s_inb, 16)
            for p in range(PAIRS):
                sync.dma_start(
                    onb[:, 2 * p:2 * p + 2, :],
                    offn[p, :, :, :].transpose([1, 0, 2]),
                ).then_inc(s_inx[p], 16)
            for cidx, (p, w0, nw) in enumerate(CHUNKS):
                sync.wait_ge(s_fin[cidx], 1)
                dst = out[p, :, w0:w0 + nw, :]
                sync.dma_start(dst, Rb[cidx % NR][:, 0:nw, :]).then_inc(s_out[cidx], 16)

        @block.vector
        def _(v: bass.BassEngine):
            ch = Chain(v, s_cv)
            r = ch.run

            def floor_anchor(sl, tsl, dn_batched):
                """Floor + anchors over onb channel slice sl; frac sub deferred."""
                onf = onb[:, sl, :]
                r(lambda: v.tensor_scalar(sy2[:, sl, :], onf, TWO23, -TWO23, OP.add, OP.add))
                r(lambda: v.tensor_tensor(sf[:, sl, :], sy2[:, sl, :], onf, OP.is_gt))
                r(lambda: v.tensor_sub(sy2[:, sl, :], sy2[:, sl, :], sf[:, sl, :]))
                npair = (sl.stop - sl.start) // 2
                hs = slice(sl.start, sl.stop, 2)
                ws = slice(sl.start + 1, sl.stop, 2)
                r(lambda: v.scalar_tensor_tensor(
                    tD[:, tsl, :], sy2[:, hs, :], float(HP), sy2[:, ws, :], OP.mult, OP.add),
                  waits=[(s_inb, 16)])
                r(lambda: v.tensor_tensor(
                    tD[:, tsl, :], tD[:, tsl, :],
                    bnat[:, :].unsqueeze(1).broadcast_to([128, npair, W]), OP.add))
                if dn_batched:
                    r(lambda: v.tensor_copy(dnat[:, tsl, :], tD[:, tsl, :]),
                      final=(s_dn[1], 1))
                else:
                    for p in range(tsl.start, tsl.stop):
                        r(lambda p=p: v.tensor_copy(dnat[:, p, :], tD[:, p, :]),
                          final=(s_dn[p], 1))
                r(lambda: v.tensor_sub(sf[:, sl, :], onf, sy2[:, sl, :]))

            # pair 0 fast path unblocks ACT wrap + first gathers ASAP
            v.wait_ge(s_inx[0], 16)
            floor_anchor(slice(0, 2), slice(0, 1), False)
            for p in range(1, PAIRS):
                v.wait_ge(s_inx[p], 16)
            floor_anchor(slice(2, 8), slice(1, 4), True)
            # weights: wt4[:, p, w, k]; k = 2*dh + dw
            r(lambda: v.tensor_scalar(sg[:, :, :], sf[:, :, :], -1.0, 1.0, OP.mult, OP.add))
            hsel = {0: sg, 1: sf}
            for kk in range(K):
                a, b = divmod(kk, 2)
                fin = (s_wt, 1) if kk == K - 1 else None
                r(lambda a=a, b=b, kk=kk: v.tensor_tensor(
                    wt4[:, :, :, kk], hsel[a][:, 0::2, :], hsel[b][:, 1::2, :], OP.mult),
                  final=fin)

            for cidx, (p, w0, nw) in enumerate(CHUNKS):
                P = Pb[cidx % NP]
                T2 = T2b[cidx % NT]
                wv = wt4[:, p, w0:w0 + nw, :].unsqueeze(2).broadcast_to(
                    [128, nw, C, K])
                waits = [(s_g[cidx], 16)]
                if cidx == 0:
                    waits.append((s_wt, 1))
                if cidx >= NP:
                    waits.append((s_t2[cidx - NP], 1))   # P buf reuse: addk done
                r(lambda P=P, wv=wv, G=Gb[cidx % NG], nw=nw: v.tensor_tensor(
                    P[:, 0:nw, :, :], G[:, 0:nw, :, :], wv, OP.mult),
                  waits=waits, final=(s_mul[cidx], 1))
                v.wait_ge(s_mul[cidx], 1)
                waits = []
                if cidx >= NT:
                    waits.append((s_fin[cidx - NT], 1))  # T2 buf reuse
                r(lambda P=P, T2=T2, nw=nw: v.tensor_tensor(
                    T2[:, 0:nw, :, :], P[:, 0:nw, :, 0:2], P[:, 0:nw, :, 2:4], OP.add),
                  waits=waits, final=(s_t2[cidx], 1))
                if final_split[cidx] == "v":
                    v.wait_ge(s_t2[cidx], 1)
                    waits = []
                    if cidx >= NR:
                        waits.append((s_out[cidx - NR], 16))
                    r(lambda T2=T2, R=Rb[cidx % NR], nw=nw: v.tensor_tensor(
                        R[:, 0:nw, :], T2[:, 0:nw, :, 0], T2[:, 0:nw, :, 1], OP.add),
                      waits=waits, final=(s_fin[cidx], 1))

        @block.scalar
        def _(act: bass.BassEngine):
            for p in range(PAIRS):
                act.memzero(db[p][:, :].bitcast(BF16)).then_inc(s_cg, 1)
            act.wait_ge(s_cg, 4)
            for p in range(PAIRS):
                act.wait_ge(s_dn[min(p, 1)], 1)
                dwrap = db[p][:, :].rearrange("q (w k) -> q w k", k=8)
                for k in range(0, 8, 2):
                    act.copy(dwrap[0:16, :, k],
                             dnat[16 * k:16 * (k + 1), p, :]).then_inc(s_dw[p], 1)
                act.wait_ge(s_dw[p], 4)
                with nc.allow_non_contiguous_dma(reason="idx-wrap strided dst"):
                    for k in (1, 3, 5, 7):
                        act.dma_start(dwrap[0:16, :, k],
                                      dnat[16 * k:16 * (k + 1), p, :]).then_inc(s_dw[p], 16)
                act.wait_ge(s_dw[p], 4 + 64)
                act.dma_start(db[p][16:32, :],
                              db[p][0:16, :]).then_inc(s_dw[p], 16)

        @block.gpsimd
        def _(g: bass.BassGpSimd):
            chg = Chain(g, s_cg)
            g.load_library(mlp)
            n_ms = 0
            pool_finals = []
            emitted = 0

            def emit_final(cidx):
                waits = [(s_t2[cidx], 1)]
                if cidx >= NR:
                    waits.append((s_out[cidx - NR], 16))
                T2 = T2b[cidx % NT]
                nw = CHUNKS[cidx][2]
                chg.run(lambda T2=T2, R=Rb[cidx % NR], nw=nw: g.tensor_tensor(
                    R[:, 0:nw, :], T2[:, 0:nw, :, 0], T2[:, 0:nw, :, 1], OP.add),
                    waits=waits, final=(s_fin[cidx], 1))

            for cidx, (p, w0, nw) in enumerate(CHUNKS):
                waits = [(s_dw[p], 4 + 64 + 16)]
                if cidx >= NG:
                    waits.append((s_mul[cidx - NG], 1))
                for sem, val in waits:
                    g.wait_ge(sem, val)
                nidx = H * nw
                g.dma_gather(
                    Gb[cidx % NG][:, 0:nw, :, :].rearrange("q w c k -> q w (c k)"),
                    patches[p, :, :],
                    db[p][:, w0 * 8:(w0 + nw) * 8],
                    nidx,
                    nidx,
                    128,
                    single_packet=False,
                ).then_inc(s_g[cidx], 16)
                # weave pool finals behind the gather stream
                while (emitted < len(pool_finals) and
                       pool_finals[emitted] <= cidx - 2):
                    emit_final(pool_finals[emitted])
                    emitted += 1
                if final_split[cidx] == "g":
                    pool_finals.append(cidx)
            while emitted < len(pool_finals):
                emit_final(pool_finals[emitted])
                emitted += 1

    nc.compile()
    return nc


# ---------------- host-side helpers ----------------

def build_patches_all(imgs_pairs):
    """(npair, C, H, W) f32 -> (npair, NROWS, 128) bf16, rows (c, k)."""
    import ml_dtypes

    npair = imgs_pairs.shape[0]
    hw_c = np.ascontiguousarray(np.transpose(imgs_pairs, (0, 2, 3, 1)))  # (n, H, W, C)
    padded = np.zeros((npair, HP + 1, HP + 1, C), np.float32)
    padded[:, PAD:PAD + H, PAD:PAD + W] = hw_c
    P = np.empty((npair, HP, HP, C, K), np.float32)
    P[:, :, :, :, 0] = padded[:, 0:HP, 0:HP]
    P[:, :, :, :, 1] = padded[:, 0:HP, 1:HP + 1]
    P[:, :, :, :, 2] = padded[:, 1:HP + 1, 0:HP]
    P[:, :, :, :, 3] = padded[:, 1:HP + 1, 1:HP + 1]
    return P.reshape(npair, NROWS, 128).astype(ml_dtypes.bfloat16)


def base_natural():
    h = np.arange(H).reshape(H, 1)
    w = np.arange(W).reshape(1, W)
    return ((h + PAD) * HP + (w + PAD)).astype(np.float32)


def make_in_map(imgs_pairs, offp):
    return {
        "patches": build_patches_all(imgs_pairs),
        "offn": np.ascontiguousarray(offp),
        "basen": base_natural(),
    }


# ---------------- public entry point ----------------

N_CORES = 8
PAIRS_TOTAL = 32

LAST_EXEC_TIME_NS = None


def kernel(images, offsets):
    import os
    global LAST_EXEC_TIME_NS
    from concourse.bass_utils import run_bass_kernel_spmd

    images = np.ascontiguousarray(np.asarray(images, dtype=np.float32))
    offsets = np.ascontiguousarray(np.asarray(offsets, dtype=np.float32))
    imgs = images.reshape(PAIRS_TOTAL, C, H, W)
    offp = offsets.reshape(4, 8, 2, H, W).reshape(PAIRS_TOTAL, 2, H, W)

    nc = build_nc()
    in_maps = []
    for core in range(N_CORES):
        sl = slice(core * PAIRS, (core + 1) * PAIRS)
        in_maps.append(make_in_map(imgs[sl], offp[sl]))
    trace = bool(os.environ.get("DK_TRACE"))
    res = run_bass_kernel_spmd(nc, in_maps, list(range(N_CORES)), trace=trace)
    if trace:
        LAST_EXEC_TIME_NS = res.exec_time_ns
        if res.instructions_and_trace:
            print("trace path:", res.instructions_and_trace[1])
    full = np.empty((PAIRS_TOTAL, C, H, W), np.float32)
    for i in range(N_CORES):
        od = np.asarray(res.results[i]["out"]).astype(np.float32)   # (4, H, W, C)
        sl = slice(i * PAIRS, (i + 1) * PAIRS)
        full[sl] = np.transpose(od, (0, 3, 1, 2))
    return np.ascontiguousarray(full.reshape(4, 8, C, H, W)).astype(np.float32)
